# revision 43
# baseline (speedup 1.0000x reference)
"""Self-contained Trainium2 kernel for ReRoPE sparse attention.

Problem: x(2,1024,2048) -> attention with 16 Q heads / 8 KV heads (GQA),
RoPE within a 256-token causal band, ReRoPE (query rotated at fixed
position 256, keys unrotated) outside the band, -> out proj (2048x2048).

Sharding: 8 cores = 2 batches x 4 head groups. Each core computes 4 Q
heads / 2 KV heads of one batch plus its slice of all projections, and
produces a partial (1024,2048) output (wo row-parallel). Partials are
summed on the host (the per-batch all-reduce equivalent).

Score identity used: s2 = (R_W q)@k  ==  q @ (R_{-W} k), so the fixed
ReRoPE rotation is applied once to K instead of Q (q2 is just raw q).
Head dims are de-interleaved (evens|odds) via a host-side permutation of
wq/wk columns so RoPE pairs live on partitions (p, p+64).

Schedule: all inputs are packed host-side into one DRAM blob in exact
consumption order (8 chunk-pair groups of x|wk|wv|wq, then masks and
rope tables, then wo) and streamed as ~22 large descriptors over both
HWDGE rings, so the PE is dense from ~12us. Wave A runs 4 K chains plus
4 V chains chunk-paced as data arrives (static PSUM split, tags 'ps'/
'sc' 4+4 banks, keeps slow-release score psums from ever blocking
Q/attn-V slots in the pool FIFO). Wave B interleaves per head: Q chain
-> rope -> wide key-stationary scores (band N<=384 + far N<=512 per key
block, exp'd on the scalar engine); the previous head's attn-V chain
for row j is emitted after key block j's scores, so the in-order PE
queue always has ready work while score psum slots wait on the
exp stream. The band/far overlap q-block is merged in place with one
copy_predicated (far where k<=q, band where q<k; uint8 mask). Each
row's four A tiles transpose through one PSUM bank into a per-row A^T
tile, and the wo out-projection + output DMA flush row-by-row so
nothing serializes at the end.

All device compute in bf16 (fp32 PSUM accumulation).
"""

import numpy as np
import ml_dtypes

B, S, D = 2, 1024, 2048
NH, NKV, HD = 16, 8, 128
W = 256
HPC, KPC = 4, 2            # q heads / kv heads per core
KC = D // 128              # 16 contraction chunks
SB = S // 128              # 8 sequence blocks
SCALE = 1.0 / float(np.sqrt(HD))
BF16 = ml_dtypes.bfloat16

# blob column offsets (bf16 [128, NBLOB])
O_GRP = 0                   # 8 groups of (x 2048 | wk 512 | wv 512 | wq 1024)
GRPW = 4096
O_SEG = 8 * GRPW            # ident | m0 | m2 | tab(2S)
O_ID = O_SEG
O_M0 = O_SEG + 128
O_M2 = O_SEG + 256
O_TAB = O_SEG + 384
O_WO = O_TAB + 2 * S        # 4 heads * 2048
NBLOB = O_WO + HPC * D

_NC_CACHE = {}


def _build_nc():
    import concourse.bass as bass
    import concourse.tile as tile
    from concourse import bacc, mybir
    from contextlib import ExitStack

    bf = mybir.dt.bfloat16
    f32 = mybir.dt.float32
    AF = mybir.ActivationFunctionType
    M = mybir.AluOpType

    nc = bacc.Bacc()
    blob = nc.declare_dram_parameter("blob", [128, NBLOB], bf, isOutput=False)
    cwd = nc.declare_dram_parameter("cw", [128, 3], f32, isOutput=False)
    m0i = nc.declare_dram_parameter("m0i", [128, 128], mybir.dt.uint8,
                                    isOutput=False)
    out = nc.declare_dram_parameter("out", [S, D], bf, isOutput=True)

    with tile.TileContext(nc) as tc:
        with ExitStack() as ctx:
            p_in = ctx.enter_context(tc.tile_pool(name="p_in", bufs=1))
            p_q = ctx.enter_context(tc.tile_pool(name="p_q", bufs=2 * HPC))
            p_k = ctx.enter_context(tc.tile_pool(name="p_k", bufs=2 * KPC))
            p_v = ctx.enter_context(tc.tile_pool(name="p_v", bufs=SB))
            p_ao = ctx.enter_context(tc.tile_pool(name="p_ao", bufs=HPC))
            p_e = ctx.enter_context(tc.tile_pool(name="p_e", bufs=16))
            p_pt = ctx.enter_context(tc.tile_pool(name="p_pt", bufs=34))
            p_rt = ctx.enter_context(tc.tile_pool(name="p_rt", bufs=4))
            p_rc = ctx.enter_context(tc.tile_pool(name="p_rc", bufs=4))
            p_st = ctx.enter_context(tc.tile_pool(name="p_st", bufs=4))

            ps = ctx.enter_context(
                tc.tile_pool(name="ps", bufs=8, space="PSUM"))

            def pst(w, name):
                """tag 'ps' (4 banks): K chains, Q chains, attn-V, the
                per-row transpose bank."""
                return ps.tile([128, w], f32, tag="ps", name=name,
                               padded_shape=[128, 512], bufs=4)

            def psc(w, name):
                """tag 'sc' (4 banks): V chains, score psums, out-proj
                psums — slow-release consumers, kept out of the Q/AV
                slot FIFO."""
                return ps.tile([128, w], f32, tag="sc", name=name,
                               padded_shape=[128, 512], bufs=4)

            # ---- input DMA: blob streamed in consumption order ----
            bsb = p_in.tile([128, NBLOB], bf, tag="blob")
            cw_sb = p_in.tile([128, 3], f32, tag="cw")
            nc.scalar.dma_start(cw_sb[:], cwd[:, :])
            m0i_sb = p_in.tile([128, 128], mybir.dt.uint8, tag="m0i")
            nc.scalar.dma_start(m0i_sb[:], m0i[:, :])
            # x halves on sync ring, weight halves on scalar ring, in
            # group order; tables after g2; wo at the end on both rings
            for g in range(8):
                o = O_GRP + g * GRPW
                if g == 0:
                    nc.sync.dma_start(bsb[:, o:o + 512],
                                      blob[:, o:o + 512])
                    nc.scalar.dma_start(bsb[:, o + 2048:o + 2304],
                                        blob[:, o + 2048:o + 2304])
                    nc.sync.dma_start(bsb[:, o + 512:o + 1024],
                                      blob[:, o + 512:o + 1024])
                    nc.scalar.dma_start(bsb[:, o + 2304:o + 3072],
                                        blob[:, o + 2304:o + 3072])
                    nc.sync.dma_start(bsb[:, o + 1024:o + 2048],
                                      blob[:, o + 1024:o + 2048])
                    nc.scalar.dma_start(bsb[:, o + 3072:o + 4096],
                                        blob[:, o + 3072:o + 4096])
                    continue
                if g == 1:
                    nc.sync.dma_start(bsb[:, o:o + 1024],
                                      blob[:, o:o + 1024])
                    nc.scalar.dma_start(bsb[:, o + 2048:o + 3072],
                                        blob[:, o + 2048:o + 3072])
                    nc.sync.dma_start(bsb[:, o + 1024:o + 2048],
                                      blob[:, o + 1024:o + 2048])
                    nc.scalar.dma_start(bsb[:, o + 3072:o + 4096],
                                        blob[:, o + 3072:o + 4096])
                    continue
                nc.sync.dma_start(bsb[:, o:o + 2048], blob[:, o:o + 2048])
                nc.scalar.dma_start(bsb[:, o + 2048:o + GRPW],
                                    blob[:, o + 2048:o + GRPW])
                if g == 2:
                    nc.sync.dma_start(bsb[:, O_SEG:O_WO],
                                      blob[:, O_SEG:O_WO])
            h_wo = O_WO + HPC * D // 2
            nc.sync.dma_start(bsb[:, O_WO:h_wo], blob[:, O_WO:h_wo])
            nc.scalar.dma_start(bsb[:, h_wo:NBLOB], blob[:, h_wo:NBLOB])

            id_t = bsb[:, O_ID:O_ID + 128]
            m0_t = bsb[:, O_M0:O_M0 + 128]       # (k <= q)
            m2_t = bsb[:, O_M2:O_M2 + 128]       # (q < k)
            cosT = bsb[:, O_TAB:O_TAB + S]
            sinT = bsb[:, O_TAB + S:O_TAB + 2 * S]  # top half negated

            def wk_c(t):      # [128, 256] chunk t of wk
                g, e = divmod(t, 2)
                o = O_GRP + g * GRPW + 2048 + e * 256
                return bsb[:, o:o + 256]

            def wv_c(t):
                g, e = divmod(t, 2)
                o = O_GRP + g * GRPW + 2560 + e * 256
                return bsb[:, o:o + 256]

            def wq_c(t):      # [128, 512] chunk t of wq
                g, e = divmod(t, 2)
                o = O_GRP + g * GRPW + 3072 + e * 512
                return bsb[:, o:o + 512]

            def xts(t, lo, hi):
                g, e = divmod(t, 2)
                o = O_GRP + g * GRPW + e * 1024
                return bsb[:, o + lo:o + hi]

            def wo_s(h, lo, hi):
                o = O_WO + h * D
                return bsb[:, o + lo:o + hi]

            def rope_var(dst, src, eng=None):
                """Positional rope on full [128, S]; pairs on (p, p+64).
                sinT carries signs: rows 0:64 = -sin, 64:128 = +sin.
                TensorTensor needs co-based SB inputs, so the half-swap
                happens in the sin-product's OUTPUT placement."""
                eng = eng or nc.vector
                tcs = p_rt.tile([128, S], bf, tag="rt")
                eng.tensor_mul(tcs[:], src[:, 0:S], cosT)
                tsw = p_rt.tile([128, S], bf, tag="rt")
                eng.tensor_mul(tsw[0:64, :], src[64:128, 0:S],
                               sinT[64:128, :])   # +qi*sin
                eng.tensor_mul(tsw[64:128, :], src[0:64, 0:S],
                               sinT[0:64, :])     # -qr*sin
                # top: qr*cos - qi*sin ; bottom: qi*cos + qr*sin
                eng.tensor_sub(dst[:, 0:S], tcs[:], tsw[:])

            def rope_negw(dst, src):
                """R_{-W}: or = r*cw + i*sw, oi = i*cw - r*sw.
                cw cols: 0 = cw, 1 = +sw, 2 = -sw (all partitions)."""
                tsw = p_rt.tile([128, S], bf, tag="rt")
                nc.vector.tensor_scalar_mul(tsw[0:64, :], src[64:128, 0:S],
                                            cw_sb[64:128, 1:2])  # ki*sw
                nc.vector.tensor_scalar_mul(tsw[64:128, :], src[0:64, 0:S],
                                            cw_sb[0:64, 2:3])    # -kr*sw
                nc.vector.scalar_tensor_tensor(
                    dst[:, 0:S], src[:, 0:S], cw_sb[:, 0:1],
                    tsw[:], M.mult, M.add)

            # ---- wave A: K chains + V chains sb0..3, chunk-paced ----
            k1_t = [p_k.tile([128, S], bf, tag="k", name=f"k1_{i}")
                    for i in range(KPC)]
            k2_t = [p_k.tile([128, S], bf, tag="k", name=f"k2_{i}")
                    for i in range(KPC)]
            kps = [pst(512, f"kps{i}") for i in range(4)]
            vps_a = [psc(256, f"vpsa{i}") for i in range(4)]
            for t in range(KC):
                for kv in range(KPC):
                    for half in range(2):
                        nc.tensor.matmul(
                            kps[kv * 2 + half][:],
                            lhsT=wk_c(t)[:, kv * 128:(kv + 1) * 128],
                            rhs=xts(t, half * 512, (half + 1) * 512),
                            start=(t == 0), stop=(t == KC - 1))
                for sb in range(4):
                    nc.tensor.matmul(
                        vps_a[sb][:],
                        lhsT=xts(t, sb * 128, (sb + 1) * 128),
                        rhs=wv_c(t),
                        start=(t == 0), stop=(t == KC - 1))

            kr_t = []
            for kv in range(KPC):
                kr = p_rt.tile([128, S], bf, tag="kr", name=f"kr{kv}")
                for half in range(2):
                    nc.scalar.copy(
                        kr[:, half * 512:(half + 1) * 512],
                        kps[kv * 2 + half][:])
                kr_t.append(kr)

            v_t = [p_v.tile([128, 2 * (HD + 1)], bf, tag="v",
                            name=f"v{sb}") for sb in range(SB)]

            def v_fin(sb, vp):
                tv = v_t[sb]
                nc.vector.tensor_copy(tv[:, 0:HD], vp[:, 0:HD])
                nc.vector.tensor_copy(tv[:, HD + 1:2 * HD + 1],
                                      vp[:, HD:2 * HD])
                nc.vector.memset(tv[:, HD:HD + 1], 1.0)
                nc.vector.memset(tv[:, 2 * HD + 1:2 * HD + 2], 1.0)

            for sb in range(4):
                v_fin(sb, vps_a[sb])
            # kv0's rope feeds head-0 scores first; kv1 can wait until
            # after the V-b copies so those release their PSUM slots
            rope_var(k1_t[0], kr_t[0])
            rope_negw(k2_t[0], kr_t[0])

            # ---- wave B: V sb3..7 + Q projections (x resident now) ----
            vps_b = [psc(256, f"vpsb{i}") for i in range(4)]
            for sb in range(4, SB):
                for t in range(KC):
                    nc.tensor.matmul(
                        vps_b[sb - 4][:],
                        lhsT=xts(t, sb * 128, (sb + 1) * 128),
                        rhs=wv_c(t),
                        start=(t == 0), stop=(t == KC - 1))
            for sb in range(4, SB):
                v_fin(sb, vps_b[sb - 4])
            rope_var(k1_t[1], kr_t[1])
            rope_negw(k2_t[1], kr_t[1])

            q1_t, q2_t = [], []

            def emit_qproj(h):
                d1 = p_q.tile([128, S], bf, tag="q", name=f"q1_{h}")
                d2 = p_q.tile([128, S], bf, tag="q", name=f"q2_{h}")
                pss = [pst(512, f"qps{h}{half}") for half in range(2)]
                for t in range(KC):
                    for half in range(2):
                        nc.tensor.matmul(
                            pss[half][:],
                            lhsT=wq_c(t)[:, h * 128:(h + 1) * 128],
                            rhs=xts(t, half * 512, (half + 1) * 512),
                            start=(t == 0), stop=(t == KC - 1))
                for half in range(2):
                    nc.scalar.copy(
                        d2[:, half * 512:(half + 1) * 512], pss[half][:])
                rope_var(d1, d2)
                q1_t.append(d1)
                q2_t.append(d2)

            # ---- attention: per head, key-stationary wide scores ----
            # aoR[i] holds row i's A^T for all 4 heads: [128hd, h*128+q]

            def emit_scores(h, after_j=None):
                """Per key block j: band tile Pb [128, nb*128] (q-blocks
                j..j+2, diag m0-masked) and far tile Pf [128, nf*128]
                (q-blocks j+2..7). The band/far overlap q-block j+2 is
                merged into Pb with one copy_predicated (far where k<=q,
                band where q<k). after_j(j) lets the caller interleave
                the previous head's attn-V chains between key blocks so
                the PE queue never stalls on exp-paced score psums."""
                kv = h // 2
                Pb, Pf = [], []
                for j in range(SB):
                    nb = min(3, SB - j)          # band q-blocks j..j+2
                    pb = p_e.tile([128, nb * 128], bf, tag="pb",
                                  name=f"Pb{h}{j}", bufs=32)
                    psb = psc(nb * 128, f"sb{h}{j}")
                    nc.tensor.matmul(
                        psb[:], lhsT=k1_t[kv][:, j * 128:(j + 1) * 128],
                        rhs=q1_t[h][:, j * 128:(j + nb) * 128],
                        start=True, stop=True)
                    nc.scalar.activation(pb[:], psb[:], AF.Exp,
                                         scale=SCALE)
                    engm = nc.vector if j % 2 == 0 else nc.gpsimd
                    engm.tensor_mul(pb[:, 0:128], pb[:, 0:128], m0_t)
                    pf = None
                    if j <= SB - 3:
                        nf = SB - j - 2
                        pf = p_e.tile([128, nf * 128], bf, tag="pf",
                                      name=f"Pf{h}{j}", bufs=24)
                        o = 0
                        while o < nf * 128:
                            wseg = min(512, nf * 128 - o)
                            psf = psc(wseg, f"sf{h}{j}{o}")
                            nc.tensor.matmul(
                                psf[:],
                                lhsT=k2_t[kv][:, j * 128:(j + 1) * 128],
                                rhs=q2_t[h][:, (j + 2) * 128 + o:
                                            (j + 2) * 128 + o + wseg],
                                start=True, stop=True)
                            nc.scalar.activation(pf[:, o:o + wseg],
                                                 psf[:], AF.Exp,
                                                 scale=SCALE)
                            o += wseg
                        # merge overlap q-block j+2 in place: far where
                        # k<=q, band where q<k
                        nc.vector.copy_predicated(pb[:, 256:384],
                                                  m0i_sb[:],
                                                  pf[:, 0:128])
                    Pb.append(pb)
                    Pf.append(pf)
                    if after_j is not None:
                        after_j(j)
                return Pb, Pf

            def emit_attnv(i, h, Pb, Pf):
                kv = h // 2
                ops = []   # (lhsT block, key j) contributions
                for j in range(i + 1):
                    d = i - j
                    if d <= 2:
                        ops.append((Pb[j][:, d * 128:(d + 1) * 128], j))
                    elif Pf[j] is not None:
                        ops.append(
                            (Pf[j][:, (d - 2) * 128:(d - 1) * 128], j))
                pso = pst(HD + 1, f"av{h}{i}")
                for m, (lh, j) in enumerate(ops):
                    nc.tensor.matmul(
                        pso[:], lhsT=lh,
                        rhs=v_t[j][:, kv * (HD + 1):(kv + 1) * (HD + 1)],
                        start=(m == 0), stop=(m == len(ops) - 1))
                rc = p_rc.tile([128, 1], f32, tag="rc")
                nc.vector.reciprocal(rc[:], pso[:, HD:HD + 1])
                an = p_pt.tile([128, 128], bf, tag="an")
                nc.vector.tensor_scalar_mul(an[:], pso[:, 0:HD], rc[:])
                return an

            def flush(row, aoR):
                for cg in range(4):
                    po = psc(512, f"out{row}{cg}")
                    for hc in range(HPC):
                        nc.tensor.matmul(
                            po[:],
                            lhsT=aoR[:, hc * 128:(hc + 1) * 128],
                            rhs=wo_s(hc, cg * 512, (cg + 1) * 512),
                            start=(hc == 0), stop=(hc == HPC - 1))
                    st = p_st.tile([128, 512], bf, tag="st")
                    if cg % 2 == 0:
                        nc.vector.tensor_copy(st[:], po[:])
                    else:
                        nc.scalar.copy(st[:], po[:])
                    nc.sync.dma_start(
                        out[row * 128:(row + 1) * 128,
                            cg * 512:(cg + 1) * 512], st[:])

            # Q proj of head h+1 is emitted BEFORE scores of head h so
            # the next Q chain's PSUM slots are allocated ahead of the
            # exp-paced score psums in the pool FIFO; exps of head h
            # overlap the Q chain of h+1 on the scalar engine, and the
            # attn-V chains of heads 0-2 fill the Q->attention
            # transition so the PE never idles waiting on exps. Only
            # the last head's attn-V runs row-major with the flushes.
            ans = [[None] * SB for _ in range(HPC)]
            Pts = [None] * HPC
            emit_qproj(0)
            emit_qproj(1)
            Pts[0] = emit_scores(0)
            emit_qproj(2)

            def av_of(h):
                def cb(j):
                    ans[h][j] = emit_attnv(j, h, *Pts[h])
                return cb
            Pts[1] = emit_scores(1, av_of(0))
            emit_qproj(3)
            Pts[2] = emit_scores(2, av_of(1))
            Pts[3] = emit_scores(3, av_of(2))
            for i in range(SB):
                ans[3][i] = emit_attnv(i, 3, *Pts[3])
                pt = ps.tile([128, 4 * 128], bf, tag="ps",
                             name=f"tr{i}", padded_shape=[128, 1024],
                             bufs=4)
                for h in range(HPC):
                    nc.tensor.transpose(pt[:, h * 128:(h + 1) * 128],
                                        ans[h][i][:], id_t)
                aoR = p_ao.tile([128, 4 * 128], bf, tag="ao",
                                name=f"aoR{i}")
                nc.vector.tensor_copy(aoR[:], pt[:])
                flush(i, aoR)

    nc.finalize()
    return nc


def _get_nc():
    if "nc" not in _NC_CACHE:
        _NC_CACHE["nc"] = _build_nc()
    return _NC_CACHE["nc"]


def _host_inputs(x, freqs_cos, freqs_sin, wq, wk, wv, wo):
    """Build the 8 per-core input maps (host-side shard + layout prep)."""
    x = np.asarray(x, np.float32)
    wq = np.asarray(wq, np.float32)
    wk = np.asarray(wk, np.float32)
    wv = np.asarray(wv, np.float32)
    wo = np.asarray(wo, np.float32)
    perm = np.concatenate([np.arange(0, HD, 2), np.arange(1, HD, 2)])

    cos_t = np.asarray(freqs_cos, np.float32).T        # (64, S)
    sin_t = np.asarray(freqs_sin, np.float32).T
    tab = np.concatenate([
        np.concatenate([cos_t, -sin_t], axis=1),       # rows 0:64
        np.concatenate([cos_t, sin_t], axis=1),        # rows 64:128
    ], axis=0)                                         # (128, 2S)
    ki = np.arange(128)[:, None]
    qi = np.arange(128)[None, :]
    m0 = (ki <= qi).astype(np.float32)                 # causal / far-select
    m2 = (qi < ki).astype(np.float32)                  # in-band select

    wq3 = wq.reshape(D, NH, HD)
    wk3 = wk.reshape(D, NKV, HD)
    wv3 = wv.reshape(D, NKV, HD)
    wo3 = wo.reshape(NH, HD, D)

    cwh = np.stack([cos_t[:, W], sin_t[:, W], -sin_t[:, W]],
                   axis=1)                              # (64, 3)
    cw = np.concatenate([cwh, cwh], axis=0).astype(np.float32)

    in_maps = []
    for c in range(8):
        b, g = divmod(c, 4)
        wqc = wq3[:, 4 * g:4 * g + 4][:, :, perm].reshape(D, HPC * HD)
        wkc = wk3[:, 2 * g:2 * g + 2][:, :, perm].reshape(D, KPC * HD)
        wvc = wv3[:, 2 * g:2 * g + 2].reshape(D, KPC * HD)
        woc = wo3[4 * g:4 * g + 4].reshape(HPC * HD, D)
        xt = x[b].T                                     # (D, S)

        blob = np.empty((128, NBLOB), np.float32)
        blob[:, O_ID:O_ID + 128] = np.eye(128, dtype=np.float32)
        blob[:, O_M0:O_M0 + 128] = m0
        blob[:, O_M2:O_M2 + 128] = m2
        blob[:, O_TAB:O_TAB + 2 * S] = tab
        for gi in range(8):
            o = O_GRP + gi * GRPW
            for e in range(2):
                t = 2 * gi + e
                rs = slice(t * 128, (t + 1) * 128)
                blob[:, o + e * 1024:o + (e + 1) * 1024] = xt[rs]
                blob[:, o + 2048 + e * 256:
                     o + 2048 + (e + 1) * 256] = wkc[rs]
                blob[:, o + 2560 + e * 256:
                     o + 2560 + (e + 1) * 256] = wvc[rs]
                blob[:, o + 3072 + e * 512:
                     o + 3072 + (e + 1) * 512] = wqc[rs]
        blob[:, O_WO:NBLOB] = woc.reshape(HPC, HD, D).transpose(
            1, 0, 2).reshape(128, HPC * D)
        in_maps.append({
            "blob": np.ascontiguousarray(blob).astype(BF16),
            "cw": cw,
            "m0i": m0.astype(np.uint8),
        })
    return in_maps


def _run(nc, in_maps, **kw):
    from concourse.bass_utils import run_bass_kernel_spmd
    return run_bass_kernel_spmd(nc, in_maps, core_ids=list(range(8)), **kw)


def kernel(x, freqs_cos, freqs_sin, wq, wk, wv, wo):
    nc = _get_nc()
    in_maps = _host_inputs(x, freqs_cos, freqs_sin, wq, wk, wv, wo)
    res = _run(nc, in_maps)
    parts = [np.asarray(res.results[c]["out"], np.float32) for c in range(8)]
    out = np.stack([sum(parts[0:4]), sum(parts[4:8])])
    return out.astype(np.float32)


# revision 44
# speedup vs baseline: 1.1723x; 1.1723x over previous
"""Self-contained Trainium2 kernel for ReRoPE sparse attention.

Problem: x(2,1024,2048) -> attention with 16 Q heads / 8 KV heads (GQA),
RoPE within a 256-token causal band, ReRoPE (query rotated at fixed
position 256, keys unrotated) outside the band, -> out proj (2048x2048).

Sharding: 8 cores = 2 batches x 4 head groups. Each core computes 4 Q
heads / 2 KV heads of one batch plus its slice of all projections, and
produces a partial (1024,2048) output (wo row-parallel). Partials are
summed on the host (the per-batch all-reduce equivalent).

Score identity used: s2 = (R_W q)@k  ==  q @ (R_{-W} k), so the fixed
ReRoPE rotation is applied once to K instead of Q (q2 is just raw q).
Head dims are de-interleaved (evens|odds) via a host-side permutation of
wq/wk columns so RoPE pairs live on partitions (p, p+64).

Schedule: all inputs are packed host-side into one DRAM blob in exact
consumption order (8 chunk-pair groups of x|wk|wv|wq, then masks and
rope tables, then wo) and streamed as ~22 large descriptors over both
HWDGE rings, so the PE is dense from ~12us. Wave A runs 4 K chains plus
4 V chains chunk-paced as data arrives (static PSUM split, tags 'ps'/
'sc' 4+4 banks, keeps slow-release score psums from ever blocking
Q/attn-V slots in the pool FIFO). Wave B interleaves per head: Q chain
-> rope -> wide key-stationary scores (band N<=384 + far N<=512 per key
block, exp'd on the scalar engine); the previous head's attn-V chain
for row j is emitted after key block j's scores, so the in-order PE
queue always has ready work while score psum slots wait on the
exp stream. The band/far overlap q-block is merged in place with one
copy_predicated (far where k<=q, band where q<k; uint8 mask). Each
row's four A tiles transpose through one PSUM bank into a per-row A^T
tile, and the wo out-projection + output DMA flush row-by-row so
nothing serializes at the end.

All device compute in bf16 (fp32 PSUM accumulation).
"""

import numpy as np
import ml_dtypes

B, S, D = 2, 1024, 2048
NH, NKV, HD = 16, 8, 128
W = 256
HPC, KPC = 4, 2            # q heads / kv heads per core
KC = D // 128              # 16 contraction chunks
SB = S // 128              # 8 sequence blocks
SCALE = 1.0 / float(np.sqrt(HD))
BF16 = ml_dtypes.bfloat16

# blob column offsets (bf16 [128, NBLOB])
O_GRP = 0                   # 8 groups of (x 2048 | wk 512 | wv 512 | wq 1024)
GRPW = 4096
O_SEG = 8 * GRPW            # ident | m0 | m2 | tab(2S)
O_ID = O_SEG
O_M0 = O_SEG + 128
O_M2 = O_SEG + 256
O_TAB = O_SEG + 384
O_WO = O_TAB + 2 * S        # 4 heads * 2048
NBLOB = O_WO + HPC * D

_NC_CACHE = {}


def _build_nc():
    import concourse.bass as bass
    import concourse.tile as tile
    from concourse import bacc, mybir
    from contextlib import ExitStack

    bf = mybir.dt.bfloat16
    f32 = mybir.dt.float32
    AF = mybir.ActivationFunctionType
    M = mybir.AluOpType

    nc = bacc.Bacc()
    blob = nc.declare_dram_parameter("blob", [128, NBLOB], bf, isOutput=False)
    cwd = nc.declare_dram_parameter("cw", [128, 3], f32, isOutput=False)
    m0i = nc.declare_dram_parameter("m0i", [128, 128], mybir.dt.uint8,
                                    isOutput=False)
    out = nc.declare_dram_parameter("out", [S, D], bf, isOutput=True)

    with tile.TileContext(nc) as tc:
        with ExitStack() as ctx:
            p_in = ctx.enter_context(tc.tile_pool(name="p_in", bufs=1))
            p_q = ctx.enter_context(tc.tile_pool(name="p_q", bufs=2 * HPC))
            p_k = ctx.enter_context(tc.tile_pool(name="p_k", bufs=2 * KPC))
            p_v = ctx.enter_context(tc.tile_pool(name="p_v", bufs=SB))
            p_ao = ctx.enter_context(tc.tile_pool(name="p_ao", bufs=HPC))
            p_e = ctx.enter_context(tc.tile_pool(name="p_e", bufs=16))
            p_pt = ctx.enter_context(tc.tile_pool(name="p_pt", bufs=34))
            p_rt = ctx.enter_context(tc.tile_pool(name="p_rt", bufs=4))
            p_rc = ctx.enter_context(tc.tile_pool(name="p_rc", bufs=4))
            p_st = ctx.enter_context(tc.tile_pool(name="p_st", bufs=4))

            ps = ctx.enter_context(
                tc.tile_pool(name="ps", bufs=8, space="PSUM"))

            def pst(w, name):
                """tag 'ps' (4 banks): K chains, Q chains, attn-V, the
                per-row transpose bank."""
                return ps.tile([128, w], f32, tag="ps", name=name,
                               padded_shape=[128, 512], bufs=4)

            def psc(w, name):
                """tag 'sc' (4 banks): V chains, score psums, out-proj
                psums — slow-release consumers, kept out of the Q/AV
                slot FIFO."""
                return ps.tile([128, w], f32, tag="sc", name=name,
                               padded_shape=[128, 512], bufs=4)

            # ---- input DMA: blob streamed in consumption order ----
            bsb = p_in.tile([128, NBLOB], bf, tag="blob")
            cw_sb = p_in.tile([128, 3], f32, tag="cw")
            nc.scalar.dma_start(cw_sb[:], cwd[:, :])
            m0i_sb = p_in.tile([128, 128], mybir.dt.uint8, tag="m0i")
            nc.scalar.dma_start(m0i_sb[:], m0i[:, :])
            # x halves on sync ring, weight halves on scalar ring, in
            # group order; tables after g2; wo at the end on both rings
            # wave A consumes only x|wk|wv: the wq columns of every
            # group are deferred to a second pass so the weight ring
            # drains in ~6us and the x stream gets full DMA bandwidth,
            # making wave A compute-bound instead of arrival-paced
            for g in range(8):
                o = O_GRP + g * GRPW
                if g == 0:
                    nc.sync.dma_start(bsb[:, o:o + 512],
                                      blob[:, o:o + 512])
                    nc.scalar.dma_start(bsb[:, o + 2048:o + 2304],
                                        blob[:, o + 2048:o + 2304])
                    nc.sync.dma_start(bsb[:, o + 512:o + 1024],
                                      blob[:, o + 512:o + 1024])
                    nc.scalar.dma_start(bsb[:, o + 2304:o + 3072],
                                        blob[:, o + 2304:o + 3072])
                    nc.sync.dma_start(bsb[:, o + 1024:o + 2048],
                                      blob[:, o + 1024:o + 2048])
                    continue
                if g == 1:
                    nc.sync.dma_start(bsb[:, o:o + 1024],
                                      blob[:, o:o + 1024])
                    nc.scalar.dma_start(bsb[:, o + 2048:o + 3072],
                                        blob[:, o + 2048:o + 3072])
                    nc.sync.dma_start(bsb[:, o + 1024:o + 2048],
                                      blob[:, o + 1024:o + 2048])
                    continue
                nc.sync.dma_start(bsb[:, o:o + 2048], blob[:, o:o + 2048])
                nc.scalar.dma_start(bsb[:, o + 2048:o + 3072],
                                    blob[:, o + 2048:o + 3072])
                if g == 2:
                    nc.sync.dma_start(bsb[:, O_SEG:O_WO],
                                      blob[:, O_SEG:O_WO])
            for g in range(8):
                o = O_GRP + g * GRPW
                nc.scalar.dma_start(bsb[:, o + 3072:o + 4096],
                                    blob[:, o + 3072:o + 4096])
            h_wo = O_WO + HPC * D // 2
            nc.sync.dma_start(bsb[:, O_WO:h_wo], blob[:, O_WO:h_wo])
            nc.scalar.dma_start(bsb[:, h_wo:NBLOB], blob[:, h_wo:NBLOB])

            id_t = bsb[:, O_ID:O_ID + 128]
            m0_t = bsb[:, O_M0:O_M0 + 128]       # (k <= q)
            m2_t = bsb[:, O_M2:O_M2 + 128]       # (q < k)
            cosT = bsb[:, O_TAB:O_TAB + S]
            sinT = bsb[:, O_TAB + S:O_TAB + 2 * S]  # top half negated

            def wk_c(t):      # [128, 256] chunk t of wk
                g, e = divmod(t, 2)
                o = O_GRP + g * GRPW + 2048 + e * 256
                return bsb[:, o:o + 256]

            def wv_c(t):
                g, e = divmod(t, 2)
                o = O_GRP + g * GRPW + 2560 + e * 256
                return bsb[:, o:o + 256]

            def wq_c(t):      # [128, 512] chunk t of wq
                g, e = divmod(t, 2)
                o = O_GRP + g * GRPW + 3072 + e * 512
                return bsb[:, o:o + 512]

            def xts(t, lo, hi):
                g, e = divmod(t, 2)
                o = O_GRP + g * GRPW + e * 1024
                return bsb[:, o + lo:o + hi]

            def wo_s(h, lo, hi):
                o = O_WO + h * D
                return bsb[:, o + lo:o + hi]

            def rope_var(dst, src, eng=None):
                """Positional rope on full [128, S]; pairs on (p, p+64).
                sinT carries signs: rows 0:64 = -sin, 64:128 = +sin.
                TensorTensor needs co-based SB inputs, so the half-swap
                happens in the sin-product's OUTPUT placement."""
                eng = eng or nc.vector
                tcs = p_rt.tile([128, S], bf, tag="rt")
                eng.tensor_mul(tcs[:], src[:, 0:S], cosT)
                tsw = p_rt.tile([128, S], bf, tag="rt")
                eng.tensor_mul(tsw[0:64, :], src[64:128, 0:S],
                               sinT[64:128, :])   # +qi*sin
                eng.tensor_mul(tsw[64:128, :], src[0:64, 0:S],
                               sinT[0:64, :])     # -qr*sin
                # top: qr*cos - qi*sin ; bottom: qi*cos + qr*sin
                eng.tensor_sub(dst[:, 0:S], tcs[:], tsw[:])

            def rope_negw(dst, src):
                """R_{-W}: or = r*cw + i*sw, oi = i*cw - r*sw.
                cw cols: 0 = cw, 1 = +sw, 2 = -sw (all partitions)."""
                tsw = p_rt.tile([128, S], bf, tag="rt")
                nc.vector.tensor_scalar_mul(tsw[0:64, :], src[64:128, 0:S],
                                            cw_sb[64:128, 1:2])  # ki*sw
                nc.vector.tensor_scalar_mul(tsw[64:128, :], src[0:64, 0:S],
                                            cw_sb[0:64, 2:3])    # -kr*sw
                nc.vector.scalar_tensor_tensor(
                    dst[:, 0:S], src[:, 0:S], cw_sb[:, 0:1],
                    tsw[:], M.mult, M.add)

            # ---- wave A: K chains + V chains sb0..3, chunk-paced ----
            k1_t = [p_k.tile([128, S], bf, tag="k", name=f"k1_{i}")
                    for i in range(KPC)]
            k2_t = [p_k.tile([128, S], bf, tag="k", name=f"k2_{i}")
                    for i in range(KPC)]
            kps = [pst(512, f"kps{i}") for i in range(4)]
            vps_a = [psc(256, f"vpsa{i}") for i in range(4)]
            for t in range(KC):
                for kv in range(KPC):
                    for half in range(2):
                        nc.tensor.matmul(
                            kps[kv * 2 + half][:],
                            lhsT=wk_c(t)[:, kv * 128:(kv + 1) * 128],
                            rhs=xts(t, half * 512, (half + 1) * 512),
                            start=(t == 0), stop=(t == KC - 1))
                for sb in range(4):
                    nc.tensor.matmul(
                        vps_a[sb][:],
                        lhsT=xts(t, sb * 128, (sb + 1) * 128),
                        rhs=wv_c(t),
                        start=(t == 0), stop=(t == KC - 1))

            kr_t = []
            for kv in range(KPC):
                kr = p_rt.tile([128, S], bf, tag="kr", name=f"kr{kv}")
                for half in range(2):
                    nc.scalar.copy(
                        kr[:, half * 512:(half + 1) * 512],
                        kps[kv * 2 + half][:])
                kr_t.append(kr)

            v_t = [p_v.tile([128, 2 * (HD + 1)], bf, tag="v",
                            name=f"v{sb}") for sb in range(SB)]

            def v_fin(sb, vp):
                tv = v_t[sb]
                nc.vector.tensor_copy(tv[:, 0:HD], vp[:, 0:HD])
                nc.vector.tensor_copy(tv[:, HD + 1:2 * HD + 1],
                                      vp[:, HD:2 * HD])
                nc.vector.memset(tv[:, HD:HD + 1], 1.0)
                nc.vector.memset(tv[:, 2 * HD + 1:2 * HD + 2], 1.0)

            for sb in range(4):
                v_fin(sb, vps_a[sb])
            # kv0's rope feeds head-0 scores first; kv1 can wait until
            # after the V-b copies so those release their PSUM slots
            rope_var(k1_t[0], kr_t[0])
            rope_negw(k2_t[0], kr_t[0])

            # ---- wave B: V sb3..7 + Q projections (x resident now) ----
            vps_b = [psc(256, f"vpsb{i}") for i in range(4)]
            for sb in range(4, SB):
                for t in range(KC):
                    nc.tensor.matmul(
                        vps_b[sb - 4][:],
                        lhsT=xts(t, sb * 128, (sb + 1) * 128),
                        rhs=wv_c(t),
                        start=(t == 0), stop=(t == KC - 1))
            for sb in range(4, SB):
                v_fin(sb, vps_b[sb - 4])
            rope_var(k1_t[1], kr_t[1])
            rope_negw(k2_t[1], kr_t[1])

            q1_t, q2_t = [], []

            def emit_qproj(h):
                d1 = p_q.tile([128, S], bf, tag="q", name=f"q1_{h}")
                d2 = p_q.tile([128, S], bf, tag="q", name=f"q2_{h}")
                pss = [pst(512, f"qps{h}{half}") for half in range(2)]
                for t in range(KC):
                    for half in range(2):
                        nc.tensor.matmul(
                            pss[half][:],
                            lhsT=wq_c(t)[:, h * 128:(h + 1) * 128],
                            rhs=xts(t, half * 512, (half + 1) * 512),
                            start=(t == 0), stop=(t == KC - 1))
                for half in range(2):
                    nc.scalar.copy(
                        d2[:, half * 512:(half + 1) * 512], pss[half][:])
                rope_var(d1, d2)
                q1_t.append(d1)
                q2_t.append(d2)

            # ---- attention: per head, key-stationary wide scores ----
            # aoR[i] holds row i's A^T for all 4 heads: [128hd, h*128+q]

            def emit_scores(h, after_j=None):
                """Per key block j: band tile Pb [128, nb*128] (q-blocks
                j..j+2, diag m0-masked) and far tile Pf [128, nf*128]
                (q-blocks j+2..7). The band/far overlap q-block j+2 is
                merged into Pb with one copy_predicated (far where k<=q,
                band where q<k). after_j(j) lets the caller interleave
                the previous head's attn-V chains between key blocks so
                the PE queue never stalls on exp-paced score psums."""
                kv = h // 2
                Pb, Pf = [], []
                for j in range(SB):
                    nb = min(3, SB - j)          # band q-blocks j..j+2
                    pb = p_e.tile([128, nb * 128], bf, tag="pb",
                                  name=f"Pb{h}{j}", bufs=32)
                    psb = psc(nb * 128, f"sb{h}{j}")
                    nc.tensor.matmul(
                        psb[:], lhsT=k1_t[kv][:, j * 128:(j + 1) * 128],
                        rhs=q1_t[h][:, j * 128:(j + nb) * 128],
                        start=True, stop=True)
                    nc.scalar.activation(pb[:], psb[:], AF.Exp,
                                         scale=SCALE)
                    engm = nc.vector if j % 2 == 0 else nc.gpsimd
                    engm.tensor_mul(pb[:, 0:128], pb[:, 0:128], m0_t)
                    pf = None
                    if j <= SB - 3:
                        nf = SB - j - 2
                        pf = p_e.tile([128, nf * 128], bf, tag="pf",
                                      name=f"Pf{h}{j}", bufs=24)
                        o = 0
                        while o < nf * 128:
                            wseg = min(512, nf * 128 - o)
                            psf = psc(wseg, f"sf{h}{j}{o}")
                            nc.tensor.matmul(
                                psf[:],
                                lhsT=k2_t[kv][:, j * 128:(j + 1) * 128],
                                rhs=q2_t[h][:, (j + 2) * 128 + o:
                                            (j + 2) * 128 + o + wseg],
                                start=True, stop=True)
                            nc.scalar.activation(pf[:, o:o + wseg],
                                                 psf[:], AF.Exp,
                                                 scale=SCALE)
                            o += wseg
                        # merge overlap q-block j+2 in place: far where
                        # k<=q, band where q<k
                        nc.vector.copy_predicated(pb[:, 256:384],
                                                  m0i_sb[:],
                                                  pf[:, 0:128])
                    Pb.append(pb)
                    Pf.append(pf)
                    if after_j is not None:
                        after_j(j)
                return Pb, Pf

            def emit_attnv(i, h, Pb, Pf):
                kv = h // 2
                ops = []   # (lhsT block, key j) contributions
                for j in range(i + 1):
                    d = i - j
                    if d <= 2:
                        ops.append((Pb[j][:, d * 128:(d + 1) * 128], j))
                    elif Pf[j] is not None:
                        ops.append(
                            (Pf[j][:, (d - 2) * 128:(d - 1) * 128], j))
                pso = pst(HD + 1, f"av{h}{i}")
                for m, (lh, j) in enumerate(ops):
                    nc.tensor.matmul(
                        pso[:], lhsT=lh,
                        rhs=v_t[j][:, kv * (HD + 1):(kv + 1) * (HD + 1)],
                        start=(m == 0), stop=(m == len(ops) - 1))
                rc = p_rc.tile([128, 1], f32, tag="rc")
                nc.vector.reciprocal(rc[:], pso[:, HD:HD + 1])
                an = p_pt.tile([128, 128], bf, tag="an")
                nc.vector.tensor_scalar_mul(an[:], pso[:, 0:HD], rc[:])
                return an

            def flush(row, aoR):
                for cg in range(4):
                    po = psc(512, f"out{row}{cg}")
                    for hc in range(HPC):
                        nc.tensor.matmul(
                            po[:],
                            lhsT=aoR[:, hc * 128:(hc + 1) * 128],
                            rhs=wo_s(hc, cg * 512, (cg + 1) * 512),
                            start=(hc == 0), stop=(hc == HPC - 1))
                    st = p_st.tile([128, 512], bf, tag="st")
                    if cg % 2 == 0:
                        nc.vector.tensor_copy(st[:], po[:])
                    else:
                        nc.scalar.copy(st[:], po[:])
                    nc.sync.dma_start(
                        out[row * 128:(row + 1) * 128,
                            cg * 512:(cg + 1) * 512], st[:])

            # Q proj of head h+1 is emitted BEFORE scores of head h so
            # the next Q chain's PSUM slots are allocated ahead of the
            # exp-paced score psums in the pool FIFO; exps of head h
            # overlap the Q chain of h+1 on the scalar engine, and the
            # attn-V chains of heads 0-2 fill the Q->attention
            # transition so the PE never idles waiting on exps. Only
            # the last head's attn-V runs row-major with the flushes.
            ans = [[None] * SB for _ in range(HPC)]
            Pts = [None] * HPC
            emit_qproj(0)
            emit_qproj(1)
            Pts[0] = emit_scores(0)
            emit_qproj(2)

            def av_of(h):
                def cb(j):
                    ans[h][j] = emit_attnv(j, h, *Pts[h])
                return cb
            Pts[1] = emit_scores(1, av_of(0))
            emit_qproj(3)
            Pts[2] = emit_scores(2, av_of(1))
            Pts[3] = emit_scores(3, av_of(2))
            for i in range(SB):
                ans[3][i] = emit_attnv(i, 3, *Pts[3])
                pt = ps.tile([128, 4 * 128], bf, tag="ps",
                             name=f"tr{i}", padded_shape=[128, 1024],
                             bufs=4)
                for h in range(HPC):
                    nc.tensor.transpose(pt[:, h * 128:(h + 1) * 128],
                                        ans[h][i][:], id_t)
                aoR = p_ao.tile([128, 4 * 128], bf, tag="ao",
                                name=f"aoR{i}")
                nc.vector.tensor_copy(aoR[:], pt[:])
                flush(i, aoR)

    nc.finalize()
    return nc


def _get_nc():
    if "nc" not in _NC_CACHE:
        _NC_CACHE["nc"] = _build_nc()
    return _NC_CACHE["nc"]


def _host_inputs(x, freqs_cos, freqs_sin, wq, wk, wv, wo):
    """Build the 8 per-core input maps (host-side shard + layout prep)."""
    x = np.asarray(x, np.float32)
    wq = np.asarray(wq, np.float32)
    wk = np.asarray(wk, np.float32)
    wv = np.asarray(wv, np.float32)
    wo = np.asarray(wo, np.float32)
    perm = np.concatenate([np.arange(0, HD, 2), np.arange(1, HD, 2)])

    cos_t = np.asarray(freqs_cos, np.float32).T        # (64, S)
    sin_t = np.asarray(freqs_sin, np.float32).T
    tab = np.concatenate([
        np.concatenate([cos_t, -sin_t], axis=1),       # rows 0:64
        np.concatenate([cos_t, sin_t], axis=1),        # rows 64:128
    ], axis=0)                                         # (128, 2S)
    ki = np.arange(128)[:, None]
    qi = np.arange(128)[None, :]
    m0 = (ki <= qi).astype(np.float32)                 # causal / far-select
    m2 = (qi < ki).astype(np.float32)                  # in-band select

    wq3 = wq.reshape(D, NH, HD)
    wk3 = wk.reshape(D, NKV, HD)
    wv3 = wv.reshape(D, NKV, HD)
    wo3 = wo.reshape(NH, HD, D)

    cwh = np.stack([cos_t[:, W], sin_t[:, W], -sin_t[:, W]],
                   axis=1)                              # (64, 3)
    cw = np.concatenate([cwh, cwh], axis=0).astype(np.float32)

    in_maps = []
    for c in range(8):
        b, g = divmod(c, 4)
        wqc = wq3[:, 4 * g:4 * g + 4][:, :, perm].reshape(D, HPC * HD)
        wkc = wk3[:, 2 * g:2 * g + 2][:, :, perm].reshape(D, KPC * HD)
        wvc = wv3[:, 2 * g:2 * g + 2].reshape(D, KPC * HD)
        woc = wo3[4 * g:4 * g + 4].reshape(HPC * HD, D)
        xt = x[b].T                                     # (D, S)

        blob = np.empty((128, NBLOB), np.float32)
        blob[:, O_ID:O_ID + 128] = np.eye(128, dtype=np.float32)
        blob[:, O_M0:O_M0 + 128] = m0
        blob[:, O_M2:O_M2 + 128] = m2
        blob[:, O_TAB:O_TAB + 2 * S] = tab
        for gi in range(8):
            o = O_GRP + gi * GRPW
            for e in range(2):
                t = 2 * gi + e
                rs = slice(t * 128, (t + 1) * 128)
                blob[:, o + e * 1024:o + (e + 1) * 1024] = xt[rs]
                blob[:, o + 2048 + e * 256:
                     o + 2048 + (e + 1) * 256] = wkc[rs]
                blob[:, o + 2560 + e * 256:
                     o + 2560 + (e + 1) * 256] = wvc[rs]
                blob[:, o + 3072 + e * 512:
                     o + 3072 + (e + 1) * 512] = wqc[rs]
        blob[:, O_WO:NBLOB] = woc.reshape(HPC, HD, D).transpose(
            1, 0, 2).reshape(128, HPC * D)
        in_maps.append({
            "blob": np.ascontiguousarray(blob).astype(BF16),
            "cw": cw,
            "m0i": m0.astype(np.uint8),
        })
    return in_maps


def _run(nc, in_maps, **kw):
    from concourse.bass_utils import run_bass_kernel_spmd
    return run_bass_kernel_spmd(nc, in_maps, core_ids=list(range(8)), **kw)


def kernel(x, freqs_cos, freqs_sin, wq, wk, wv, wo):
    nc = _get_nc()
    in_maps = _host_inputs(x, freqs_cos, freqs_sin, wq, wk, wv, wo)
    res = _run(nc, in_maps)
    parts = [np.asarray(res.results[c]["out"], np.float32) for c in range(8)]
    out = np.stack([sum(parts[0:4]), sum(parts[4:8])])
    return out.astype(np.float32)
